# revision 29
# baseline (speedup 1.0000x reference)
"""
AQ (additive-quantization) expert layer on 8 TRN2 NeuronCores.

  out = clip((x * scales) @ W.T, -50, 50)
  W[o, g*8+j] = sum_c codebooks[c, indices[o, g, c], j]

Strategy: 2x4 mesh (2 token-halves x 4 out-feature quarters).
  - Core (it, io) owns tokens [it*4096, +4096) and out features
    [io*1024, +1024).  Per-core matmul work is identical to pure
    tensor-parallel (17.2 G MACs), but per-core HBM traffic drops from
    151 MB to 92 MB, so the DMA path (x stream at ~330 GB/s) has ~2x
    slack over the PE floor instead of running razor-thin.
  - Host-side prep is layout-only (byte movement, no float arithmetic):
    x pre-transposed to x^T [K, T] f32 and split in token halves; the two
    codebook contributions laid out as W^T-shaped bf16 panels per out-
    quarter (shared by both token-half cores); scales per (partition, kc).
  - On device, per core:
      panels: gt0 DMA'd straight into the resident W^T buffer in
              kc-eighth chunks (sync queue); gt1 streamed through a
              2-deep staging ring on the scalar queue; DVE merges
              wt[:, kc, :] = (gt0 + gt1) * s per kc as chunks land, so
              the PE can start after ~one chunk instead of a full panel.
      x:      streamed 512-token groups in kc-quarter chunks with SWDGE
              f32->bf16 cast (gpsimd queue); per-quarter semaphores let
              the first matmul start after 1/4 of a tile.
      matmul: out^T = W @ x^T accumulated over 32 kc per (t-group,
              o-block); 8 o-blocks map to the 8 PSUM banks; per-o-block
              eviction waits make bank turnaround free at t-group
              boundaries; DVE clips on PSUM evict; out^T DMA'd out.
  - Host reassembles the 8 [1024, 4096] out^T shards into [T, OUT].
"""

import sys

sys.path.insert(0, "/opt/trn_rl_repo")

import numpy as np
import ml_dtypes

from concourse import bass, mybir
from concourse.bass_utils import run_bass_kernel_spmd

F32 = mybir.dt.float32
BF16 = mybir.dt.bfloat16

N_CORES = 8
M_TOK = 2                                  # token-parallel ways
M_OUT = 4                                  # out-feature-parallel ways
GS = 8
NCB = 2
CBS = 256

FULL_CFG = dict(T=8192, IN_F=4096, OUT_F=4096)


def _cfg(T, IN_F, OUT_F):
    cfg = {}
    cfg["T"] = T
    cfg["IN_F"] = IN_F
    cfg["OUT_F"] = OUT_F
    cfg["TSH"] = T // M_TOK                # tokens per core
    cfg["OSH"] = OUT_F // M_OUT            # out-features per core
    cfg["KC"] = IN_F // 128                # number of 128-wide k-chunks
    cfg["TGT"] = min(512, cfg["TSH"])      # tokens per t-group
    cfg["NTG"] = cfg["TSH"] // cfg["TGT"]
    cfg["OB"] = cfg["OSH"] // 128          # 128-wide o-blocks per core
    cfg["NE"] = 8                          # gt panel chunks (kc-eighths)
    cfg["EKC"] = cfg["KC"] // cfg["NE"]    # kc per panel chunk
    cfg["NXQ"] = 4                         # x chunks per t-group
    cfg["QKC"] = cfg["KC"] // cfg["NXQ"]   # kc per x chunk
    assert cfg["OB"] <= 8                  # one PSUM bank per o-block
    return cfg


def ap(t, off, dims):
    return bass.AP(t, off, dims)


def build_nc(cfg):
    TSH, KC, OSH = cfg["TSH"], cfg["KC"], cfg["OSH"]
    TGT, NTG, OB = cfg["TGT"], cfg["NTG"], cfg["OB"]
    NE, EKC, NXQ, QKC = cfg["NE"], cfg["EKC"], cfg["NXQ"], cfg["QKC"]
    IN_F = cfg["IN_F"]

    nc = bass.Bass(target_bir_lowering=False)

    # x^T pre-tiled on host to [NTG, 128, KC, TGT]: each x-tile DMA is
    # 128 long contiguous runs -> 8x fewer SWDGE descriptors
    xt = nc.declare_dram_parameter(
        "xt", [NTG * 128, KC * TGT], F32, isOutput=False
    )
    gt0 = nc.declare_dram_parameter("gt0", [128, KC * OSH], BF16, isOutput=False)
    gt1 = nc.declare_dram_parameter("gt1", [128, KC * OSH], BF16, isOutput=False)
    sc = nc.declare_dram_parameter("sc", [128, KC], F32, isOutput=False)
    outT = nc.declare_dram_parameter("outT", [OSH, TSH], BF16, isOutput=True)

    n_tiles_total = NTG * OB
    NSTG = 8                               # out staging ring depth

    import contextlib

    with contextlib.ExitStack() as stack:
        en = stack.enter_context
        s_w = en(nc.semaphore("s_w"))       # sc load done
        s_g = [en(nc.semaphore(f"s_g{e}")) for e in range(NE)]  # panel chunks
        s_tm = en(nc.semaphore("s_tm"))     # merge adds done (1 per kc)
        s_wt = en(nc.semaphore("s_wt"))     # Wt chunks scaled (1 per kc)
        s_xq = [en(nc.semaphore(f"s_xq{q}")) for q in range(NXQ)]  # tg0 x
        s_xr = [en(nc.semaphore(f"s_xr{i}")) for i in range(3)]    # tg>=1 x
        s_mm = en(nc.semaphore("s_mm"))     # psum tiles finished (1 per tile)
        s_ev = en(nc.semaphore("s_ev"))     # psum tiles evicted (DVE only)
        # per-staging-slot out-DMA completion (slot == ob, epoch == tg);
        # per-slot counting is immune to cross-DMA completion reordering
        s_sl = [en(nc.semaphore(f"s_sl{k}")) for k in range(NSTG)]
        s_wu = en(nc.semaphore("s_wu"))     # SWDGE warm-up (unused)

        wt_sb = en(nc.sbuf_tensor("wt_sb", [128, KC * OSH], BF16))
        g1s_sb = en(nc.sbuf_tensor("g1s_sb", [128, 4 * EKC * OSH], BF16))
        sc_sb = en(nc.sbuf_tensor("sc_sb", [128, KC], F32))
        stg_sb = en(nc.sbuf_tensor("stg_sb", [128, NSTG * 512], BF16))
        xtbs = [
            en(nc.sbuf_tensor(f"xtb{i}_sb", [128, KC * TGT], BF16))
            for i in range(3)
        ]
        psums = [
            en(nc.psum_tensor(f"ps{b}", [128, 512], F32)) for b in range(OB)
        ]

        def out_dma(engine, tile):
            tg, ob = tile // OB, tile % OB
            engine.dma_start(
                ap(outT, (ob * 128) * TSH + tg * TGT, [[TSH, 128], [1, TGT]]),
                ap(stg_sb, (tile % NSTG) * 512, [[NSTG * 512, 128], [1, TGT]]),
            ).then_inc(s_sl[tile % NSTG], 16)

        with nc.Block() as blk:

            @blk.sync
            def _(sync):
                sync.dma_start(sc_sb[:, :], sc[:, :]).then_inc(s_w, 16)
                # gt0 on the sync HW queue (~180 GB/s), gt1 on the scalar
                # HW queue, x on the SWDGE FIFO — three channels in
                # parallel so no stream waits behind another
                n = EKC * OSH
                for e in range(NE):
                    sync.dma_start(
                        ap(wt_sb, e * n, [[KC * OSH, 128], [1, n]]),
                        ap(gt0, e * n, [[KC * OSH, 128], [1, n]]),
                    ).then_inc(s_g[e], 16)
                # even-tile out DMAs
                for tile in range(0, n_tiles_total, 2):
                    sync.wait_ge(s_ev, tile + 1)
                    out_dma(sync, tile)
                for k in range(NSTG):
                    sync.wait_ge(s_sl[k], 16 * (n_tiles_total // NSTG))

            @blk.scalar
            def _(scalar):
                n = EKC * OSH
                for e in range(NE):
                    if e >= 4:
                        # g1 staging slot e%4 free once adds of e-4 done
                        scalar.wait_ge(s_tm, EKC * (e - 3))
                    scalar.dma_start(
                        ap(g1s_sb, (e % 4) * n, [[4 * n, 128], [1, n]]),
                        ap(gt1, e * n, [[KC * OSH, 128], [1, n]]),
                    ).then_inc(s_g[e], 16)
                # scale-muls follow the DVE merge adds kc by kc
                for kc in range(KC):
                    scalar.wait_ge(s_tm, kc + 1)
                    scalar.activation(
                        ap(wt_sb, kc * OSH, [[KC * OSH, 128], [1, OSH]]),
                        ap(wt_sb, kc * OSH, [[KC * OSH, 128], [1, OSH]]),
                        mybir.ActivationFunctionType.Copy,
                        scale=ap(sc_sb, kc, [[KC, 128], [1, 1]]),
                    ).then_inc(s_wt, 1)
                # odd-tile out DMAs
                for tile in range(1, n_tiles_total, 2):
                    scalar.wait_ge(s_ev, tile + 1)
                    out_dma(scalar, tile)

            @blk.gpsimd
            def _(gpsimd):
                # SWDGE FIFO: gt1 chunks and the early x stream interleave
                # in PE-need order on one queue (x needs the f32->bf16
                # cast anyway); gt0 rides the parallel sync HW queue.
                # tg0 and tg1 x go in kc-quarters on the four quarter sems
                # (epochs 1 and 2) so the PE starts each on a quarter.
                gpsimd.dma_start(
                    ap(stg_sb, 0, [[NSTG * 512, 128], [1, 512]]),
                    ap(xt, 0, [[KC * TGT, 128], [1, 512]]),
                ).then_inc(s_wu, 16)

                def x_load(tg, q=None, sem=None):
                    if q is None:
                        kc0, nkc = 0, KC
                    else:
                        kc0, nkc = q * QKC, QKC
                    gpsimd.dma_start(
                        ap(
                            xtbs[tg % 3],
                            kc0 * TGT,
                            [[KC * TGT, 128], [1, nkc * TGT]],
                        ),
                        ap(
                            xt,
                            (tg * 128 * KC + kc0) * TGT,
                            [[KC * TGT, 128], [1, nkc * TGT]],
                        ),
                    ).then_inc(sem, 16)

                for q in range(NXQ):
                    x_load(0, q=q, sem=s_xq[q])
                for q in range(NXQ):
                    x_load(1, q=q, sem=s_xq[q])
                for tg in range(2, NTG):
                    if tg >= 3:
                        # buffer tg%3 free once tg-2 fully computed
                        gpsimd.wait_ge(s_mm, OB * (tg - 2))
                    x_load(tg, sem=s_xr[(tg - 2) % 3])

            # DVE: merge adds wt += gt1, then all psum evicts
            @blk.vector
            def _(vector):
                vector.wait_ge(s_w, 16)
                for kc in range(KC):
                    e = kc // EKC
                    if kc % EKC == 0:
                        vector.wait_ge(s_g[e], 32)
                    vector.tensor_add(
                        ap(wt_sb, kc * OSH, [[KC * OSH, 128], [1, OSH]]),
                        ap(wt_sb, kc * OSH, [[KC * OSH, 128], [1, OSH]]),
                        ap(
                            g1s_sb,
                            (e % 4) * EKC * OSH + (kc % EKC) * OSH,
                            [[4 * EKC * OSH, 128], [1, OSH]],
                        ),
                    ).then_inc(s_tm, 1)
                for tile in range(n_tiles_total):
                    vector.wait_ge(s_mm, tile + 1)
                    if tile >= NSTG:
                        # staging slot free once all prior outs of this
                        # slot have drained
                        vector.wait_ge(
                            s_sl[tile % NSTG], 16 * (tile // NSTG)
                        )
                    vector.tensor_scalar(
                        ap(
                            stg_sb,
                            (tile % NSTG) * 512,
                            [[NSTG * 512, 128], [1, TGT]],
                        ),
                        ap(psums[tile % OB], 0, [[512, 128], [1, TGT]]),
                        50.0,
                        -50.0,
                        mybir.AluOpType.min,
                        mybir.AluOpType.max,
                    ).then_inc(s_ev, 1)

            @blk.tensor
            def _(tensor):
                wt_ap = lambda kc, ob: ap(
                    wt_sb, kc * OSH + ob * 128, [[KC * OSH, 128], [1, 128]]
                )

                # tg0/tg1: kc-outer so matmuls start after the first x
                # quarter of each (the ramp is DMA-bound; fine-grain x
                # waits keep the PE fed as quarters land)
                for tg in (0, 1):
                    xsb = xtbs[tg]
                    for kc in range(KC):
                        if kc % QKC == 0:
                            tensor.wait_ge(s_xq[kc // QKC], 16 * (tg + 1))
                        if tg == 0:
                            tensor.wait_ge(s_wt, kc + 1)
                        for ob in range(OB):
                            if tg == 1 and kc == 0:
                                # bank ob free once (tg0, ob) evicted
                                tensor.wait_ge(s_ev, ob + 1)
                            inst = tensor.matmul(
                                ap(psums[ob], 0, [[512, 128], [1, TGT]]),
                                wt_ap(kc, ob),
                                ap(
                                    xsb,
                                    kc * TGT,
                                    [[KC * TGT, 128], [1, TGT]],
                                ),
                                start=(kc == 0),
                                stop=(kc == KC - 1),
                            )
                            if kc == KC - 1:
                                inst.then_inc(s_mm, 1)

                # tg>=2: ob-outer so each tile completes ~7 us apart,
                # giving evictions a whole-tile window (no boundary stalls)
                for tg in range(2, NTG):
                    xsb = xtbs[tg % 3]
                    tensor.wait_ge(s_xr[(tg - 2) % 3], 16 * ((tg - 2) // 3 + 1))
                    for ob in range(OB):
                        # bank ob free once (tg-1, ob) evicted
                        tensor.wait_ge(s_ev, (tg - 1) * OB + ob + 1)
                        for kc in range(KC):
                            inst = tensor.matmul(
                                ap(psums[ob], 0, [[512, 128], [1, TGT]]),
                                wt_ap(kc, ob),
                                ap(
                                    xsb,
                                    kc * TGT,
                                    [[KC * TGT, 128], [1, TGT]],
                                ),
                                start=(kc == 0),
                                stop=(kc == KC - 1),
                            )
                            if kc == KC - 1:
                                inst.then_inc(s_mm, 1)

    return nc


# ------------------- host-side prep (layout only) -------------------

def prep_inputs(x, indices, codebooks, scales, cfg):
    """Pure layout/packing transforms; all arithmetic happens on device."""
    T, IN_F, OUT_F = cfg["T"], cfg["IN_F"], cfg["OUT_F"]
    TSH, OSH, KC = cfg["TSH"], cfg["OSH"], cfg["KC"]

    NTG, TGT = cfg["NTG"], cfg["TGT"]
    x2d = np.asarray(x, dtype=np.float32).reshape(T, IN_F)
    # tile to [NTG, 128(p), KC, TGT]: x_tiled[tg, p, kc, t] =
    # x2d[tg*TGT + t, kc*128 + p]  (pure layout)
    xts = []
    for it in range(M_TOK):
        xh = x2d[it * TSH : (it + 1) * TSH]
        xh = xh.reshape(NTG, TGT, KC, 128).transpose(0, 3, 2, 1)
        xts.append(np.ascontiguousarray(xh).reshape(NTG * 128, KC * TGT))

    idx = np.asarray(indices)  # [OUT_F, G, 2]
    cb = np.asarray(codebooks, dtype=ml_dtypes.bfloat16)  # [2, 256, 8]

    scales = np.asarray(scales, dtype=np.float32)
    sc = np.ascontiguousarray(scales.reshape(KC, 128).T)  # [128, KC]

    # per out-quarter panels, shared by both token-half cores
    panels = []
    for io in range(M_OUT):
        ci = idx[io * OSH : (io + 1) * OSH]  # [OSH, G, 2]
        pair = []
        for c in range(NCB):
            # gt_c[k, o] = cb[c, ci[o, k//8, c], k%8]  (byte placement only)
            g = cb[c][ci[:, :, c]]                  # [OSH, G, 8]
            g = g.reshape(OSH, IN_F).T              # [IN_F, OSH]
            g = np.ascontiguousarray(
                g.reshape(KC, 128, OSH).transpose(1, 0, 2)
            ).reshape(128, KC * OSH)
            pair.append(g)
        panels.append(pair)

    in_maps = []
    for core in range(N_CORES):
        it, io = core // M_OUT, core % M_OUT
        in_maps.append(
            {
                "xt": xts[it],
                "sc": sc,
                "gt0": panels[io][0],
                "gt1": panels[io][1],
            }
        )
    return in_maps


def _ensure_ntff_hook():
    """bass_utils' trace path imports antenv.axon_hooks, which this image
    lacks; synthesize it around trn_agent_boot's ctypes hook."""
    import types

    try:
        import antenv.axon_hooks  # noqa: F401

        return
    except ImportError:
        pass
    try:
        import antenv
    except ImportError:
        return
    m = types.ModuleType("antenv.axon_hooks")
    state = {}

    def set_axon_ntff_profile_hook(h):
        state["h"] = h

    def get_axon_ntff_profile_hook():
        if "h" not in state:
            try:
                from trn_agent_boot.trn_boot import _ntff_profile_via_ctypes

                state["h"] = _ntff_profile_via_ctypes("/opt/axon/libaxon_pjrt.so")
            except Exception:
                return None
        return state["h"]

    m.set_axon_ntff_profile_hook = set_axon_ntff_profile_hook
    m.get_axon_ntff_profile_hook = get_axon_ntff_profile_hook
    sys.modules["antenv.axon_hooks"] = m
    antenv.axon_hooks = m


def run(x, indices, codebooks, scales, cfg=None, trace=False):
    cfg = _cfg(**(cfg or FULL_CFG))
    if trace:
        _ensure_ntff_hook()
    nc = build_nc(cfg)
    in_maps = prep_inputs(x, indices, codebooks, scales, cfg)
    res = run_bass_kernel_spmd(
        nc, in_maps, core_ids=list(range(N_CORES)), trace=trace
    )
    T, OUT_F = cfg["T"], cfg["OUT_F"]
    TSH, OSH = cfg["TSH"], cfg["OSH"]
    out = np.empty((T, OUT_F), dtype=np.float32)
    for core in range(N_CORES):
        it, io = core // M_OUT, core % M_OUT
        shard = res.results[core]["outT"]  # [OSH, TSH] bf16
        out[it * TSH : (it + 1) * TSH, io * OSH : (io + 1) * OSH] = (
            shard.T.astype(np.float32)
        )
    return out, res


def kernel(x, indices, codebooks, scales):
    cfg = _cfg(**FULL_CFG)
    out2d, _ = run(x, indices, codebooks, scales)
    return out2d.reshape(4, 2048, cfg["OUT_F"]).astype(np.float32)


# revision 30
# speedup vs baseline: 1.0114x; 1.0114x over previous
"""
AQ (additive-quantization) expert layer on 8 TRN2 NeuronCores.

  out = clip((x * scales) @ W.T, -50, 50)
  W[o, g*8+j] = sum_c codebooks[c, indices[o, g, c], j]

Strategy: 2x4 mesh (2 token-halves x 4 out-feature quarters).
  - Core (it, io) owns tokens [it*4096, +4096) and out features
    [io*1024, +1024).  Per-core matmul work is identical to pure
    tensor-parallel (17.2 G MACs), but per-core HBM traffic drops from
    151 MB to 92 MB, so the DMA path (x stream at ~330 GB/s) has ~2x
    slack over the PE floor instead of running razor-thin.
  - Host-side prep is layout-only (byte movement, no float arithmetic):
    x pre-transposed to x^T [K, T] f32 and split in token halves; the two
    codebook contributions laid out as W^T-shaped bf16 panels per out-
    quarter (shared by both token-half cores); scales per (partition, kc).
  - On device, per core:
      panels: gt0 DMA'd straight into the resident W^T buffer in
              kc-eighth chunks (sync queue); gt1 streamed through a
              2-deep staging ring on the scalar queue; DVE merges
              wt[:, kc, :] = (gt0 + gt1) * s per kc as chunks land, so
              the PE can start after ~one chunk instead of a full panel.
      x:      streamed 512-token groups in kc-quarter chunks with SWDGE
              f32->bf16 cast (gpsimd queue); per-quarter semaphores let
              the first matmul start after 1/4 of a tile.
      matmul: out^T = W @ x^T accumulated over 32 kc per (t-group,
              o-block); 8 o-blocks map to the 8 PSUM banks; per-o-block
              eviction waits make bank turnaround free at t-group
              boundaries; DVE clips on PSUM evict; out^T DMA'd out.
  - Host reassembles the 8 [1024, 4096] out^T shards into [T, OUT].
"""

import sys

sys.path.insert(0, "/opt/trn_rl_repo")

import numpy as np
import ml_dtypes

from concourse import bass, mybir
from concourse.bass_utils import run_bass_kernel_spmd

F32 = mybir.dt.float32
BF16 = mybir.dt.bfloat16

N_CORES = 8
M_TOK = 2                                  # token-parallel ways
M_OUT = 4                                  # out-feature-parallel ways
GS = 8
NCB = 2
CBS = 256

FULL_CFG = dict(T=8192, IN_F=4096, OUT_F=4096)


def _cfg(T, IN_F, OUT_F):
    cfg = {}
    cfg["T"] = T
    cfg["IN_F"] = IN_F
    cfg["OUT_F"] = OUT_F
    cfg["TSH"] = T // M_TOK                # tokens per core
    cfg["OSH"] = OUT_F // M_OUT            # out-features per core
    cfg["KC"] = IN_F // 128                # number of 128-wide k-chunks
    cfg["TGT"] = min(512, cfg["TSH"])      # tokens per t-group
    cfg["NTG"] = cfg["TSH"] // cfg["TGT"]
    cfg["OB"] = cfg["OSH"] // 128          # 128-wide o-blocks per core
    cfg["NE"] = 8                          # gt panel chunks (kc-eighths)
    cfg["EKC"] = cfg["KC"] // cfg["NE"]    # kc per panel chunk
    cfg["NXQ"] = 4                         # x chunks per t-group
    cfg["QKC"] = cfg["KC"] // cfg["NXQ"]   # kc per x chunk
    assert cfg["OB"] <= 8                  # one PSUM bank per o-block
    return cfg


def ap(t, off, dims):
    return bass.AP(t, off, dims)


def build_nc(cfg):
    TSH, KC, OSH = cfg["TSH"], cfg["KC"], cfg["OSH"]
    TGT, NTG, OB = cfg["TGT"], cfg["NTG"], cfg["OB"]
    NE, EKC, NXQ, QKC = cfg["NE"], cfg["EKC"], cfg["NXQ"], cfg["QKC"]
    IN_F = cfg["IN_F"]

    nc = bass.Bass(target_bir_lowering=False)

    # x^T pre-tiled on host to [NTG, 128, KC, TGT]: each x-tile DMA is
    # 128 long contiguous runs -> 8x fewer SWDGE descriptors
    xt = nc.declare_dram_parameter(
        "xt", [NTG * 128, KC * TGT], F32, isOutput=False
    )
    gt0 = nc.declare_dram_parameter("gt0", [128, KC * OSH], BF16, isOutput=False)
    gt1 = nc.declare_dram_parameter("gt1", [128, KC * OSH], BF16, isOutput=False)
    sc = nc.declare_dram_parameter("sc", [128, KC], F32, isOutput=False)
    outT = nc.declare_dram_parameter("outT", [OSH, TSH], BF16, isOutput=True)

    n_tiles_total = NTG * OB
    NSTG = 8                               # out staging ring depth

    import contextlib

    with contextlib.ExitStack() as stack:
        en = stack.enter_context
        s_w = en(nc.semaphore("s_w"))       # sc load done
        s_g = [en(nc.semaphore(f"s_g{e}")) for e in range(NE)]  # panel chunks
        s_tm = en(nc.semaphore("s_tm"))     # merge adds done (1 per kc)
        s_wt = en(nc.semaphore("s_wt"))     # Wt chunks scaled (1 per kc)
        s_xq = [en(nc.semaphore(f"s_xq{q}")) for q in range(NXQ)]  # tg0 x
        s_xr = [en(nc.semaphore(f"s_xr{i}")) for i in range(3)]    # tg>=1 x
        s_mm = en(nc.semaphore("s_mm"))     # psum tiles finished (1 per tile)
        s_ev = en(nc.semaphore("s_ev"))     # psum tiles evicted (DVE only)
        # per-staging-slot out-DMA completion (slot == ob, epoch == tg);
        # per-slot counting is immune to cross-DMA completion reordering
        s_sl = [en(nc.semaphore(f"s_sl{k}")) for k in range(NSTG)]
        s_wu = en(nc.semaphore("s_wu"))     # SWDGE warm-up (unused)

        wt_sb = en(nc.sbuf_tensor("wt_sb", [128, KC * OSH], BF16))
        g1s_sb = en(nc.sbuf_tensor("g1s_sb", [128, 4 * EKC * OSH], BF16))
        sc_sb = en(nc.sbuf_tensor("sc_sb", [128, KC], F32))
        stg_sb = en(nc.sbuf_tensor("stg_sb", [128, NSTG * 512], BF16))
        xtbs = [
            en(nc.sbuf_tensor(f"xtb{i}_sb", [128, KC * TGT], BF16))
            for i in range(3)
        ]
        psums = [
            en(nc.psum_tensor(f"ps{b}", [128, 512], F32)) for b in range(OB)
        ]

        def out_dma(engine, tile):
            tg, ob = tile // OB, tile % OB
            engine.dma_start(
                ap(outT, (ob * 128) * TSH + tg * TGT, [[TSH, 128], [1, TGT]]),
                ap(stg_sb, (tile % NSTG) * 512, [[NSTG * 512, 128], [1, TGT]]),
            ).then_inc(s_sl[tile % NSTG], 16)

        with nc.Block() as blk:

            @blk.sync
            def _(sync):
                sync.dma_start(sc_sb[:, :], sc[:, :]).then_inc(s_w, 16)
                # gt0 on the sync HW queue (~180 GB/s), gt1 on the scalar
                # HW queue, x on the SWDGE FIFO — three channels in
                # parallel so no stream waits behind another
                n = EKC * OSH
                for e in range(NE):
                    sync.dma_start(
                        ap(wt_sb, e * n, [[KC * OSH, 128], [1, n]]),
                        ap(gt0, e * n, [[KC * OSH, 128], [1, n]]),
                    ).then_inc(s_g[e], 16)
                # even-tile out DMAs
                for tile in range(0, n_tiles_total, 2):
                    sync.wait_ge(s_ev, tile + 1)
                    out_dma(sync, tile)
                for k in range(NSTG):
                    sync.wait_ge(s_sl[k], 16 * (n_tiles_total // NSTG))

            @blk.scalar
            def _(scalar):
                n = EKC * OSH

                def gt1_load(e):
                    scalar.dma_start(
                        ap(g1s_sb, (e % 4) * n, [[4 * n, 128], [1, n]]),
                        ap(gt1, e * n, [[KC * OSH, 128], [1, n]]),
                    ).then_inc(s_g[e], 16)

                for e in range(4):
                    gt1_load(e)
                # scale-muls follow the DVE merge adds kc by kc; the late
                # gt1 chunks issue inside the loop where their staging-ring
                # guard (adds of e-4 done) is implied by program position
                for kc in range(KC):
                    scalar.wait_ge(s_tm, kc + 1)
                    scalar.activation(
                        ap(wt_sb, kc * OSH, [[KC * OSH, 128], [1, OSH]]),
                        ap(wt_sb, kc * OSH, [[KC * OSH, 128], [1, OSH]]),
                        mybir.ActivationFunctionType.Copy,
                        scale=ap(sc_sb, kc, [[KC, 128], [1, 1]]),
                    ).then_inc(s_wt, 1)
                    if kc % EKC == EKC - 1 and kc // EKC + 4 < NE:
                        gt1_load(kc // EKC + 4)
                # odd-tile out DMAs
                for tile in range(1, n_tiles_total, 2):
                    scalar.wait_ge(s_ev, tile + 1)
                    out_dma(scalar, tile)

            @blk.gpsimd
            def _(gpsimd):
                # SWDGE FIFO: gt1 chunks and the early x stream interleave
                # in PE-need order on one queue (x needs the f32->bf16
                # cast anyway); gt0 rides the parallel sync HW queue.
                # tg0 and tg1 x go in kc-quarters on the four quarter sems
                # (epochs 1 and 2) so the PE starts each on a quarter.
                gpsimd.dma_start(
                    ap(stg_sb, 0, [[NSTG * 512, 128], [1, 512]]),
                    ap(xt, 0, [[KC * TGT, 128], [1, 512]]),
                ).then_inc(s_wu, 16)

                def x_load(tg, q=None, sem=None):
                    if q is None:
                        kc0, nkc = 0, KC
                    else:
                        kc0, nkc = q * QKC, QKC
                    gpsimd.dma_start(
                        ap(
                            xtbs[tg % 3],
                            kc0 * TGT,
                            [[KC * TGT, 128], [1, nkc * TGT]],
                        ),
                        ap(
                            xt,
                            (tg * 128 * KC + kc0) * TGT,
                            [[KC * TGT, 128], [1, nkc * TGT]],
                        ),
                    ).then_inc(sem, 16)

                for q in range(NXQ):
                    x_load(0, q=q, sem=s_xq[q])
                for q in range(NXQ):
                    x_load(1, q=q, sem=s_xq[q])
                for tg in range(2, NTG):
                    if tg >= 3:
                        # buffer tg%3 free once tg-2 fully computed
                        gpsimd.wait_ge(s_mm, OB * (tg - 2))
                    x_load(tg, sem=s_xr[(tg - 2) % 3])

            # DVE: merge adds wt += gt1, then all psum evicts
            @blk.vector
            def _(vector):
                vector.wait_ge(s_w, 16)
                for kc in range(KC):
                    e = kc // EKC
                    if kc % EKC == 0:
                        vector.wait_ge(s_g[e], 32)
                    vector.tensor_add(
                        ap(wt_sb, kc * OSH, [[KC * OSH, 128], [1, OSH]]),
                        ap(wt_sb, kc * OSH, [[KC * OSH, 128], [1, OSH]]),
                        ap(
                            g1s_sb,
                            (e % 4) * EKC * OSH + (kc % EKC) * OSH,
                            [[4 * EKC * OSH, 128], [1, OSH]],
                        ),
                    ).then_inc(s_tm, 1)
                for tile in range(n_tiles_total):
                    vector.wait_ge(s_mm, tile + 1)
                    if tile >= NSTG:
                        # staging slot free once all prior outs of this
                        # slot have drained
                        vector.wait_ge(
                            s_sl[tile % NSTG], 16 * (tile // NSTG)
                        )
                    vector.tensor_scalar(
                        ap(
                            stg_sb,
                            (tile % NSTG) * 512,
                            [[NSTG * 512, 128], [1, TGT]],
                        ),
                        ap(psums[tile % OB], 0, [[512, 128], [1, TGT]]),
                        50.0,
                        -50.0,
                        mybir.AluOpType.min,
                        mybir.AluOpType.max,
                    ).then_inc(s_ev, 1)

            @blk.tensor
            def _(tensor):
                wt_ap = lambda kc, ob: ap(
                    wt_sb, kc * OSH + ob * 128, [[KC * OSH, 128], [1, 128]]
                )

                # tg0/tg1: kc-outer so matmuls start after the first x
                # quarter of each (the ramp is DMA-bound; fine-grain x
                # waits keep the PE fed as quarters land)
                for tg in (0, 1):
                    xsb = xtbs[tg]
                    for kc in range(KC):
                        if kc % QKC == 0:
                            tensor.wait_ge(s_xq[kc // QKC], 16 * (tg + 1))
                        if tg == 0:
                            tensor.wait_ge(s_wt, kc + 1)
                        for ob in range(OB):
                            if tg == 1 and kc == 0:
                                # bank ob free once (tg0, ob) evicted
                                tensor.wait_ge(s_ev, ob + 1)
                            inst = tensor.matmul(
                                ap(psums[ob], 0, [[512, 128], [1, TGT]]),
                                wt_ap(kc, ob),
                                ap(
                                    xsb,
                                    kc * TGT,
                                    [[KC * TGT, 128], [1, TGT]],
                                ),
                                start=(kc == 0),
                                stop=(kc == KC - 1),
                            )
                            if kc == KC - 1:
                                inst.then_inc(s_mm, 1)

                # tg>=2: ob-outer so each tile completes ~7 us apart,
                # giving evictions a whole-tile window (no boundary stalls)
                for tg in range(2, NTG):
                    xsb = xtbs[tg % 3]
                    tensor.wait_ge(s_xr[(tg - 2) % 3], 16 * ((tg - 2) // 3 + 1))
                    for ob in range(OB):
                        # bank ob free once (tg-1, ob) evicted
                        tensor.wait_ge(s_ev, (tg - 1) * OB + ob + 1)
                        for kc in range(KC):
                            inst = tensor.matmul(
                                ap(psums[ob], 0, [[512, 128], [1, TGT]]),
                                wt_ap(kc, ob),
                                ap(
                                    xsb,
                                    kc * TGT,
                                    [[KC * TGT, 128], [1, TGT]],
                                ),
                                start=(kc == 0),
                                stop=(kc == KC - 1),
                            )
                            if kc == KC - 1:
                                inst.then_inc(s_mm, 1)

    return nc


# ------------------- host-side prep (layout only) -------------------

def prep_inputs(x, indices, codebooks, scales, cfg):
    """Pure layout/packing transforms; all arithmetic happens on device."""
    T, IN_F, OUT_F = cfg["T"], cfg["IN_F"], cfg["OUT_F"]
    TSH, OSH, KC = cfg["TSH"], cfg["OSH"], cfg["KC"]

    NTG, TGT = cfg["NTG"], cfg["TGT"]
    x2d = np.asarray(x, dtype=np.float32).reshape(T, IN_F)
    # tile to [NTG, 128(p), KC, TGT]: x_tiled[tg, p, kc, t] =
    # x2d[tg*TGT + t, kc*128 + p]  (pure layout)
    xts = []
    for it in range(M_TOK):
        xh = x2d[it * TSH : (it + 1) * TSH]
        xh = xh.reshape(NTG, TGT, KC, 128).transpose(0, 3, 2, 1)
        xts.append(np.ascontiguousarray(xh).reshape(NTG * 128, KC * TGT))

    idx = np.asarray(indices)  # [OUT_F, G, 2]
    cb = np.asarray(codebooks, dtype=ml_dtypes.bfloat16)  # [2, 256, 8]

    scales = np.asarray(scales, dtype=np.float32)
    sc = np.ascontiguousarray(scales.reshape(KC, 128).T)  # [128, KC]

    # per out-quarter panels, shared by both token-half cores
    panels = []
    for io in range(M_OUT):
        ci = idx[io * OSH : (io + 1) * OSH]  # [OSH, G, 2]
        pair = []
        for c in range(NCB):
            # gt_c[k, o] = cb[c, ci[o, k//8, c], k%8]  (byte placement only)
            g = cb[c][ci[:, :, c]]                  # [OSH, G, 8]
            g = g.reshape(OSH, IN_F).T              # [IN_F, OSH]
            g = np.ascontiguousarray(
                g.reshape(KC, 128, OSH).transpose(1, 0, 2)
            ).reshape(128, KC * OSH)
            pair.append(g)
        panels.append(pair)

    in_maps = []
    for core in range(N_CORES):
        it, io = core // M_OUT, core % M_OUT
        in_maps.append(
            {
                "xt": xts[it],
                "sc": sc,
                "gt0": panels[io][0],
                "gt1": panels[io][1],
            }
        )
    return in_maps


def _ensure_ntff_hook():
    """bass_utils' trace path imports antenv.axon_hooks, which this image
    lacks; synthesize it around trn_agent_boot's ctypes hook."""
    import types

    try:
        import antenv.axon_hooks  # noqa: F401

        return
    except ImportError:
        pass
    try:
        import antenv
    except ImportError:
        return
    m = types.ModuleType("antenv.axon_hooks")
    state = {}

    def set_axon_ntff_profile_hook(h):
        state["h"] = h

    def get_axon_ntff_profile_hook():
        if "h" not in state:
            try:
                from trn_agent_boot.trn_boot import _ntff_profile_via_ctypes

                state["h"] = _ntff_profile_via_ctypes("/opt/axon/libaxon_pjrt.so")
            except Exception:
                return None
        return state["h"]

    m.set_axon_ntff_profile_hook = set_axon_ntff_profile_hook
    m.get_axon_ntff_profile_hook = get_axon_ntff_profile_hook
    sys.modules["antenv.axon_hooks"] = m
    antenv.axon_hooks = m


def run(x, indices, codebooks, scales, cfg=None, trace=False):
    cfg = _cfg(**(cfg or FULL_CFG))
    if trace:
        _ensure_ntff_hook()
    nc = build_nc(cfg)
    in_maps = prep_inputs(x, indices, codebooks, scales, cfg)
    res = run_bass_kernel_spmd(
        nc, in_maps, core_ids=list(range(N_CORES)), trace=trace
    )
    T, OUT_F = cfg["T"], cfg["OUT_F"]
    TSH, OSH = cfg["TSH"], cfg["OSH"]
    out = np.empty((T, OUT_F), dtype=np.float32)
    for core in range(N_CORES):
        it, io = core // M_OUT, core % M_OUT
        shard = res.results[core]["outT"]  # [OSH, TSH] bf16
        out[it * TSH : (it + 1) * TSH, io * OSH : (io + 1) * OSH] = (
            shard.T.astype(np.float32)
        )
    return out, res


def kernel(x, indices, codebooks, scales):
    cfg = _cfg(**FULL_CFG)
    out2d, _ = run(x, indices, codebooks, scales)
    return out2d.reshape(4, 2048, cfg["OUT_F"]).astype(np.float32)


# revision 34
# speedup vs baseline: 1.0447x; 1.0329x over previous
"""
AQ (additive-quantization) expert layer on 8 TRN2 NeuronCores.

  out = clip((x * scales) @ W.T, -50, 50)
  W[o, g*8+j] = sum_c codebooks[c, indices[o, g, c], j]

Strategy: 2x4 mesh (2 token-halves x 4 out-feature quarters).
  - Core (it, io) owns tokens [it*4096, +4096) and out features
    [io*1024, +1024).  Per-core matmul work is identical to pure
    tensor-parallel (17.2 G MACs = 437 us PE floor at bf16), but
    per-core HBM traffic drops from 151 MB to 92 MB so the DMA path has
    ~2x slack over the PE in steady state; the remaining bottleneck is
    the ramp (weight panels + first x tiles share one DMA engine).
  - Host-side prep is layout-only (byte movement, no float arithmetic):
    x is pre-tiled to [t-group, partition, kc, token] f32 so every
    x-tile DMA is 128 long contiguous runs (8x fewer SWDGE descriptors,
    which kills the software-DGE cold-start latency); the two codebook
    contributions are laid out as W^T-shaped bf16 panels per out-quarter
    (shared by both token-half cores); scales per (partition, kc).
  - On device, per core:
      panels: gt0 + the first half of gt1 ride the sync HW queue (gt0
              straight into the resident W^T buffer, gt1 into a 4-slot
              staging ring); the second half of gt1 interleaves with the
              x stream on the gpsimd SWDGE FIFO in PE-need order.  DVE
              adds wt += gt1 per kc as chunks land; the scalar/ACT
              engine applies the per-k scales (Copy activation with a
              per-partition scale AP), so the DVE add ladder and the
              scale ladder pipeline.
      x:      SWDGE f32->bf16 cast stream (gpsimd FIFO).  t-groups 0
              and 1 arrive in kc-quarters on four dedicated semaphores
              (epochs 1 and 2 - counting all 16 slices of a single DMA
              is reorder-safe), so the PE starts on 1/4 tile; later
              t-groups are whole-tile loads on rotating semaphores.
      matmul: out^T = W @ x^T accumulated over 32 kc per (t-group,
              o-block); 8 o-blocks map to the 8 PSUM banks.  t-groups
              0/1 run kc-outer (fine-grain ramp); t-groups 2+ run
              ob-outer so each PSUM tile completes ~7 us apart and
              evictions (DVE clip to bf16) get a whole-tile window --
              zero PE stalls at t-group boundaries.
      out:    staged through an 8-slot SBUF ring; even tiles drain on
              the sync queue, odd tiles on the scalar queue; per-slot
              semaphores (slot == o-block, epoch == t-group) make the
              ring reuse immune to cross-DMA completion reordering.
  - Host reassembles the 8 [1024, 4096] out^T shards into [T, OUT].

Measured: 490 us on 8 axon TRN2 cores (PE active ~444 us, zero matmul
gaps after the first; ramp ~26 us to first matmul, DMA-ramp bound).
"""

import sys

sys.path.insert(0, "/opt/trn_rl_repo")

import numpy as np
import ml_dtypes

from concourse import bass, mybir
from concourse.bass_utils import run_bass_kernel_spmd

F32 = mybir.dt.float32
BF16 = mybir.dt.bfloat16

N_CORES = 8
M_TOK = 2                                  # token-parallel ways
M_OUT = 4                                  # out-feature-parallel ways
GS = 8
NCB = 2
CBS = 256

FULL_CFG = dict(T=8192, IN_F=4096, OUT_F=4096)


def _cfg(T, IN_F, OUT_F):
    cfg = {}
    cfg["T"] = T
    cfg["IN_F"] = IN_F
    cfg["OUT_F"] = OUT_F
    cfg["TSH"] = T // M_TOK                # tokens per core
    cfg["OSH"] = OUT_F // M_OUT            # out-features per core
    cfg["KC"] = IN_F // 128                # number of 128-wide k-chunks
    cfg["TGT"] = min(512, cfg["TSH"])      # tokens per t-group
    cfg["NTG"] = cfg["TSH"] // cfg["TGT"]
    cfg["OB"] = cfg["OSH"] // 128          # 128-wide o-blocks per core
    cfg["NE"] = 8                          # gt panel chunks (kc-eighths)
    cfg["EKC"] = cfg["KC"] // cfg["NE"]    # kc per panel chunk
    cfg["NXQ"] = 4                         # x chunks per t-group
    cfg["QKC"] = cfg["KC"] // cfg["NXQ"]   # kc per x chunk
    assert cfg["OB"] <= 8                  # one PSUM bank per o-block
    return cfg


def ap(t, off, dims):
    return bass.AP(t, off, dims)


def build_nc(cfg):
    TSH, KC, OSH = cfg["TSH"], cfg["KC"], cfg["OSH"]
    TGT, NTG, OB = cfg["TGT"], cfg["NTG"], cfg["OB"]
    NE, EKC, NXQ, QKC = cfg["NE"], cfg["EKC"], cfg["NXQ"], cfg["QKC"]
    IN_F = cfg["IN_F"]

    nc = bass.Bass(target_bir_lowering=False)

    # x^T pre-tiled on host to [NTG, 128, KC, TGT]: each x-tile DMA is
    # 128 long contiguous runs -> 8x fewer SWDGE descriptors
    xt = nc.declare_dram_parameter(
        "xt", [NTG * 128, KC * TGT], F32, isOutput=False
    )
    gt0 = nc.declare_dram_parameter("gt0", [128, KC * OSH], BF16, isOutput=False)
    gt1 = nc.declare_dram_parameter("gt1", [128, KC * OSH], BF16, isOutput=False)
    sc = nc.declare_dram_parameter("sc", [128, KC], F32, isOutput=False)
    outT = nc.declare_dram_parameter("outT", [OSH, TSH], BF16, isOutput=True)

    n_tiles_total = NTG * OB
    NSTG = 8                               # out staging ring depth

    import contextlib

    with contextlib.ExitStack() as stack:
        en = stack.enter_context
        s_w = en(nc.semaphore("s_w"))       # sc load done
        s_g = [en(nc.semaphore(f"s_g{e}")) for e in range(NE)]  # panel chunks
        s_tm = en(nc.semaphore("s_tm"))     # merge adds done (1 per kc)
        s_wt = en(nc.semaphore("s_wt"))     # Wt chunks scaled (1 per kc)
        s_xq = [en(nc.semaphore(f"s_xq{q}")) for q in range(NXQ)]  # tg0 x
        s_xr = [en(nc.semaphore(f"s_xr{i}")) for i in range(3)]    # tg>=1 x
        s_mm = en(nc.semaphore("s_mm"))     # psum tiles finished (1 per tile)
        s_ev = en(nc.semaphore("s_ev"))     # psum tiles evicted (DVE only)
        # per-staging-slot out-DMA completion (slot == ob, epoch == tg);
        # per-slot counting is immune to cross-DMA completion reordering
        s_sl = [en(nc.semaphore(f"s_sl{k}")) for k in range(NSTG)]
        s_wu = en(nc.semaphore("s_wu"))     # SWDGE warm-up (unused)

        wt_sb = en(nc.sbuf_tensor("wt_sb", [128, KC * OSH], BF16))
        g1s_sb = en(nc.sbuf_tensor("g1s_sb", [128, 4 * EKC * OSH], BF16))
        sc_sb = en(nc.sbuf_tensor("sc_sb", [128, KC], F32))
        stg_sb = en(nc.sbuf_tensor("stg_sb", [128, NSTG * 512], BF16))
        xtbs = [
            en(nc.sbuf_tensor(f"xtb{i}_sb", [128, KC * TGT], BF16))
            for i in range(3)
        ]
        psums = [
            en(nc.psum_tensor(f"ps{b}", [128, 512], F32)) for b in range(OB)
        ]

        def out_dma(engine, tile):
            tg, ob = tile // OB, tile % OB
            engine.dma_start(
                ap(outT, (ob * 128) * TSH + tg * TGT, [[TSH, 128], [1, TGT]]),
                ap(stg_sb, (tile % NSTG) * 512, [[NSTG * 512, 128], [1, TGT]]),
            ).then_inc(s_sl[tile % NSTG], 16)

        with nc.Block() as blk:

            @blk.sync
            def _(sync):
                sync.dma_start(sc_sb[:, :], sc[:, :]).then_inc(s_w, 16)
                # gt0 (all) + gt1-0..3 ride the HW queue, which runs in
                # parallel with the SWDGE FIFO; the early gt1 chunks land
                # in staging slots 0..3 before any ring reuse
                n = EKC * OSH

                def gt0_load(e):
                    sync.dma_start(
                        ap(wt_sb, e * n, [[KC * OSH, 128], [1, n]]),
                        ap(gt0, e * n, [[KC * OSH, 128], [1, n]]),
                    ).then_inc(s_g[e], 16)

                for e in range(4):
                    sync.dma_start(
                        ap(g1s_sb, e * n, [[4 * n, 128], [1, n]]),
                        ap(gt1, e * n, [[KC * OSH, 128], [1, n]]),
                    ).then_inc(s_g[e], 16)
                    gt0_load(e)
                for e in range(4, NE):
                    gt0_load(e)
                # even-tile out DMAs
                for tile in range(0, n_tiles_total, 2):
                    sync.wait_ge(s_ev, tile + 1)
                    out_dma(sync, tile)
                for k in range(NSTG):
                    sync.wait_ge(s_sl[k], 16 * (n_tiles_total // NSTG))

            @blk.scalar
            def _(scalar):
                # scale-muls follow the DVE merge adds kc by kc
                for kc in range(KC):
                    scalar.wait_ge(s_tm, kc + 1)
                    scalar.activation(
                        ap(wt_sb, kc * OSH, [[KC * OSH, 128], [1, OSH]]),
                        ap(wt_sb, kc * OSH, [[KC * OSH, 128], [1, OSH]]),
                        mybir.ActivationFunctionType.Copy,
                        scale=ap(sc_sb, kc, [[KC, 128], [1, 1]]),
                    ).then_inc(s_wt, 1)
                # odd-tile out DMAs
                for tile in range(1, n_tiles_total, 2):
                    scalar.wait_ge(s_ev, tile + 1)
                    out_dma(scalar, tile)

            @blk.gpsimd
            def _(gpsimd):
                # SWDGE FIFO: gt1 chunks and the early x stream interleave
                # in PE-need order on one queue (x needs the f32->bf16
                # cast anyway); gt0 rides the parallel sync HW queue.
                # tg0 and tg1 x go in kc-quarters on the four quarter sems
                # (epochs 1 and 2) so the PE starts each on a quarter.
                gpsimd.dma_start(
                    ap(stg_sb, 0, [[NSTG * 512, 128], [1, 512]]),
                    ap(xt, 0, [[KC * TGT, 128], [1, 512]]),
                ).then_inc(s_wu, 16)

                def x_load(tg, q=None, sem=None):
                    if q is None:
                        kc0, nkc = 0, KC
                    else:
                        kc0, nkc = q * QKC, QKC
                    gpsimd.dma_start(
                        ap(
                            xtbs[tg % 3],
                            kc0 * TGT,
                            [[KC * TGT, 128], [1, nkc * TGT]],
                        ),
                        ap(
                            xt,
                            (tg * 128 * KC + kc0) * TGT,
                            [[KC * TGT, 128], [1, nkc * TGT]],
                        ),
                    ).then_inc(sem, 16)

                def gt1_load(e):
                    n = EKC * OSH
                    # g1 staging slot e%4 free once adds of e-4 done
                    gpsimd.wait_ge(s_tm, EKC * (e - 3))
                    gpsimd.dma_start(
                        ap(g1s_sb, (e % 4) * n, [[4 * n, 128], [1, n]]),
                        ap(gt1, e * n, [[KC * OSH, 128], [1, n]]),
                    ).then_inc(s_g[e], 16)

                x_load(0, q=0, sem=s_xq[0])
                x_load(0, q=1, sem=s_xq[1])
                gt1_load(4)
                x_load(0, q=2, sem=s_xq[2])
                gt1_load(5)
                gt1_load(6)
                x_load(0, q=3, sem=s_xq[3])
                gt1_load(7)
                for q in range(NXQ):
                    x_load(1, q=q, sem=s_xq[q])
                for tg in range(2, NTG):
                    if tg >= 3:
                        # buffer tg%3 free once tg-2 fully computed
                        gpsimd.wait_ge(s_mm, OB * (tg - 2))
                    x_load(tg, sem=s_xr[(tg - 2) % 3])

            # DVE: merge adds wt += gt1, then all psum evicts
            @blk.vector
            def _(vector):
                vector.wait_ge(s_w, 16)
                for kc in range(KC):
                    e = kc // EKC
                    if kc % EKC == 0:
                        vector.wait_ge(s_g[e], 32)
                    vector.tensor_add(
                        ap(wt_sb, kc * OSH, [[KC * OSH, 128], [1, OSH]]),
                        ap(wt_sb, kc * OSH, [[KC * OSH, 128], [1, OSH]]),
                        ap(
                            g1s_sb,
                            (e % 4) * EKC * OSH + (kc % EKC) * OSH,
                            [[4 * EKC * OSH, 128], [1, OSH]],
                        ),
                    ).then_inc(s_tm, 1)
                for tile in range(n_tiles_total):
                    vector.wait_ge(s_mm, tile + 1)
                    if tile >= NSTG:
                        # staging slot free once all prior outs of this
                        # slot have drained
                        vector.wait_ge(
                            s_sl[tile % NSTG], 16 * (tile // NSTG)
                        )
                    vector.tensor_scalar(
                        ap(
                            stg_sb,
                            (tile % NSTG) * 512,
                            [[NSTG * 512, 128], [1, TGT]],
                        ),
                        ap(psums[tile % OB], 0, [[512, 128], [1, TGT]]),
                        50.0,
                        -50.0,
                        mybir.AluOpType.min,
                        mybir.AluOpType.max,
                    ).then_inc(s_ev, 1)

            @blk.tensor
            def _(tensor):
                wt_ap = lambda kc, ob: ap(
                    wt_sb, kc * OSH + ob * 128, [[KC * OSH, 128], [1, 128]]
                )

                # tg0/tg1: kc-outer so matmuls start after the first x
                # quarter of each (the ramp is DMA-bound; fine-grain x
                # waits keep the PE fed as quarters land)
                for tg in (0, 1):
                    xsb = xtbs[tg]
                    for kc in range(KC):
                        if kc % QKC == 0:
                            tensor.wait_ge(s_xq[kc // QKC], 16 * (tg + 1))
                        if tg == 0:
                            tensor.wait_ge(s_wt, kc + 1)
                        for ob in range(OB):
                            if tg == 1 and kc == 0:
                                # bank ob free once (tg0, ob) evicted
                                tensor.wait_ge(s_ev, ob + 1)
                            inst = tensor.matmul(
                                ap(psums[ob], 0, [[512, 128], [1, TGT]]),
                                wt_ap(kc, ob),
                                ap(
                                    xsb,
                                    kc * TGT,
                                    [[KC * TGT, 128], [1, TGT]],
                                ),
                                start=(kc == 0),
                                stop=(kc == KC - 1),
                            )
                            if kc == KC - 1:
                                inst.then_inc(s_mm, 1)

                # tg>=2: ob-outer so each tile completes ~7 us apart,
                # giving evictions a whole-tile window (no boundary stalls)
                for tg in range(2, NTG):
                    xsb = xtbs[tg % 3]
                    tensor.wait_ge(s_xr[(tg - 2) % 3], 16 * ((tg - 2) // 3 + 1))
                    for ob in range(OB):
                        # bank ob free once (tg-1, ob) evicted
                        tensor.wait_ge(s_ev, (tg - 1) * OB + ob + 1)
                        for kc in range(KC):
                            inst = tensor.matmul(
                                ap(psums[ob], 0, [[512, 128], [1, TGT]]),
                                wt_ap(kc, ob),
                                ap(
                                    xsb,
                                    kc * TGT,
                                    [[KC * TGT, 128], [1, TGT]],
                                ),
                                start=(kc == 0),
                                stop=(kc == KC - 1),
                            )
                            if kc == KC - 1:
                                inst.then_inc(s_mm, 1)

    return nc


# ------------------- host-side prep (layout only) -------------------

def prep_inputs(x, indices, codebooks, scales, cfg):
    """Pure layout/packing transforms; all arithmetic happens on device."""
    T, IN_F, OUT_F = cfg["T"], cfg["IN_F"], cfg["OUT_F"]
    TSH, OSH, KC = cfg["TSH"], cfg["OSH"], cfg["KC"]

    NTG, TGT = cfg["NTG"], cfg["TGT"]
    x2d = np.asarray(x, dtype=np.float32).reshape(T, IN_F)
    # tile to [NTG, 128(p), KC, TGT]: x_tiled[tg, p, kc, t] =
    # x2d[tg*TGT + t, kc*128 + p]  (pure layout)
    xts = []
    for it in range(M_TOK):
        xh = x2d[it * TSH : (it + 1) * TSH]
        xh = xh.reshape(NTG, TGT, KC, 128).transpose(0, 3, 2, 1)
        xts.append(np.ascontiguousarray(xh).reshape(NTG * 128, KC * TGT))

    idx = np.asarray(indices)  # [OUT_F, G, 2]
    cb = np.asarray(codebooks, dtype=ml_dtypes.bfloat16)  # [2, 256, 8]

    scales = np.asarray(scales, dtype=np.float32)
    sc = np.ascontiguousarray(scales.reshape(KC, 128).T)  # [128, KC]

    # per out-quarter panels, shared by both token-half cores
    panels = []
    for io in range(M_OUT):
        ci = idx[io * OSH : (io + 1) * OSH]  # [OSH, G, 2]
        pair = []
        for c in range(NCB):
            # gt_c[k, o] = cb[c, ci[o, k//8, c], k%8]  (byte placement only)
            g = cb[c][ci[:, :, c]]                  # [OSH, G, 8]
            g = g.reshape(OSH, IN_F).T              # [IN_F, OSH]
            g = np.ascontiguousarray(
                g.reshape(KC, 128, OSH).transpose(1, 0, 2)
            ).reshape(128, KC * OSH)
            pair.append(g)
        panels.append(pair)

    in_maps = []
    for core in range(N_CORES):
        it, io = core // M_OUT, core % M_OUT
        in_maps.append(
            {
                "xt": xts[it],
                "sc": sc,
                "gt0": panels[io][0],
                "gt1": panels[io][1],
            }
        )
    return in_maps


def _ensure_ntff_hook():
    """bass_utils' trace path imports antenv.axon_hooks, which this image
    lacks; synthesize it around trn_agent_boot's ctypes hook."""
    import types

    try:
        import antenv.axon_hooks  # noqa: F401

        return
    except ImportError:
        pass
    try:
        import antenv
    except ImportError:
        return
    m = types.ModuleType("antenv.axon_hooks")
    state = {}

    def set_axon_ntff_profile_hook(h):
        state["h"] = h

    def get_axon_ntff_profile_hook():
        if "h" not in state:
            try:
                from trn_agent_boot.trn_boot import _ntff_profile_via_ctypes

                state["h"] = _ntff_profile_via_ctypes("/opt/axon/libaxon_pjrt.so")
            except Exception:
                return None
        return state["h"]

    m.set_axon_ntff_profile_hook = set_axon_ntff_profile_hook
    m.get_axon_ntff_profile_hook = get_axon_ntff_profile_hook
    sys.modules["antenv.axon_hooks"] = m
    antenv.axon_hooks = m


def run(x, indices, codebooks, scales, cfg=None, trace=False):
    cfg = _cfg(**(cfg or FULL_CFG))
    if trace:
        _ensure_ntff_hook()
    nc = build_nc(cfg)
    in_maps = prep_inputs(x, indices, codebooks, scales, cfg)
    res = run_bass_kernel_spmd(
        nc, in_maps, core_ids=list(range(N_CORES)), trace=trace
    )
    T, OUT_F = cfg["T"], cfg["OUT_F"]
    TSH, OSH = cfg["TSH"], cfg["OSH"]
    out = np.empty((T, OUT_F), dtype=np.float32)
    for core in range(N_CORES):
        it, io = core // M_OUT, core % M_OUT
        shard = res.results[core]["outT"]  # [OSH, TSH] bf16
        out[it * TSH : (it + 1) * TSH, io * OSH : (io + 1) * OSH] = (
            shard.T.astype(np.float32)
        )
    return out, res


def kernel(x, indices, codebooks, scales):
    cfg = _cfg(**FULL_CFG)
    out2d, _ = run(x, indices, codebooks, scales)
    return out2d.reshape(4, 2048, cfg["OUT_F"]).astype(np.float32)
